# revision 35
# baseline (speedup 1.0000x reference)
"""CategorySpecificLinear Trainium2 kernel.

out[t] = x[t] @ weight[category_id[t]] + bias[category_id[t]]

Strategy: expert-parallel over the 8 categories (C == n_cores == 8).
Host routes tokens by category, transposes each category's token block
to [D, T_pad] and casts x/w to bf16 (fp32 accumulate in PSUM keeps the
rel err ~3e-3, far under the 2e-2 gate). Core c computes
    out = xT.T @ w + bias    (out in bf16, host casts back to fp32)

vs the fp32r baseline (44.2 us -> 34.7 us measured):
  - bf16 halves HBM traffic (3.4 MB/core vs 9.2) and matmul cost
    (N=512 warm matmul spacing 216 ns vs 231, LDWEIGHTS ~95 ns).
  - pass A holds 8 (m, n) psum groups (all banks) k-outer, so its
    ~1.73 us per-k-step burn rate stays above the ~1.1 us/slice DMA
    delivery and the PE runs gap-free; pass B's 2 groups reuse the
    first-drained banks. ~12 bass warm-up matmuls (~2.6 us of PE
    activity) lift the HAM clock gate to 8/8 right as k=0 lands.
  - x/w slice loads rotate over 3 issuing engines (2 HWDGE + SWDGE) —
    with 2 queues the ~0.65 us per-DMA issue cost, not HBM bandwidth,
    limits delivery. The 512 KB host-tiled bias load is issued last so
    it transfers after all x/w slices (needed ~6 us later); when bias
    is all zero it is skipped and the psum->obuf drain alternates
    DVE tensor_copy / ACT copy to run two-wide.
  - out is one contiguous [m, 1024] bf16 DMA per m-tile, small
    remainder tile last, so the post-matmul tail is ~2 us.
Fixed costs outside kernel control: ~1 us framework head and ~8.6 us
postamble (per-semaphore wind-down emitted by the NEFF wrapper).

Also tried, no better: a flipped orientation (psum = w_slice.T @ x,
token remainder as narrow matmuls sharing stationary weights) measured
the same within noise (34.9 us); k-pair-batched loads, w0 split across
queues, and same-queue k0 loads all regressed (DMA issue/packet-rate
effects dominate); gpsimd tensor ops on a PSUM source fail NEFF
compile.
"""

import contextlib
import ctypes
import os
import sys
import types

import numpy as np
import ml_dtypes

sys.path.insert(0, "/opt/trn_rl_repo")

BF16 = np.dtype(ml_dtypes.bfloat16)


def _ensure_ntff_hook():
    """Provide antenv.axon_hooks if the image lacks it.

    concourse.bass_utils imports antenv.axon_hooks.get_axon_ntff_profile_hook
    when trace=True under axon; some agent images don't ship that module, in
    which case the boot's NTFF hook registration silently degrades and the
    import in bass_utils crashes. Recreate the slim ctypes hook here
    (mirrors trn_agent_boot.trn_boot._ntff_profile_via_ctypes).
    """
    try:
        import antenv.axon_hooks  # noqa: F401

        return
    except ImportError:
        pass

    so_path = "/opt/axon/libaxon_pjrt.so"
    hook = None
    if os.path.exists(so_path):
        lib = ctypes.CDLL(so_path)
        if hasattr(lib, "axon_start_nrt_profile"):
            lib.axon_start_nrt_profile.argtypes = [
                ctypes.POINTER(ctypes.c_int64),
                ctypes.c_size_t,
            ]
            lib.axon_start_nrt_profile.restype = ctypes.c_int64
            lib.axon_stop_nrt_profile.argtypes = [ctypes.c_char_p]
            lib.axon_stop_nrt_profile.restype = ctypes.c_int64

            @contextlib.contextmanager
            def hook(output_dir, device_ids):
                import jax

                jax.devices()
                if device_ids:
                    ids = (ctypes.c_int64 * len(device_ids))(*device_ids)
                    rc = lib.axon_start_nrt_profile(ids, len(device_ids))
                else:
                    rc = lib.axon_start_nrt_profile(None, 0)
                if rc != 0:
                    raise RuntimeError(f"axon_start_nrt_profile rc={rc}")
                try:
                    yield
                finally:
                    n = lib.axon_stop_nrt_profile(str(output_dir).encode())
                    if n <= 0:
                        print(
                            f"ntff profile: rc={n} writing {output_dir}",
                            file=sys.stderr,
                        )

    mod = types.ModuleType("antenv.axon_hooks")
    _state = {"hook": hook}
    mod.set_axon_ntff_profile_hook = lambda h: _state.__setitem__("hook", h)
    mod.get_axon_ntff_profile_hook = lambda: _state["hook"]
    sys.modules["antenv.axon_hooks"] = mod
    try:
        import antenv

        antenv.axon_hooks = mod
    except ImportError:
        pass


_ensure_ntff_hook()

import concourse.bass as bass
import concourse.bacc as bacc_mod
import concourse.mybir as mybir
import concourse.tile as tile
from concourse.bass import ts
from concourse.bass_utils import run_bass_kernel_spmd

N_CORES = 8
P = 128
N_TILE = 512  # one fp32 PSUM bank

_nc_cache = {}
LAST_RESULTS = None  # BassKernelResults of the most recent run (for test.py)


def _build_nc(T_pad: int, D: int, O: int, bias_is_zero: bool = False):
    KO = D // P
    NO = O // N_TILE
    bf16 = mybir.dt.bfloat16
    f32 = mybir.dt.float32

    # m-tiles: full 128-row tiles plus one remainder tile (multiple of 32)
    m_sizes = [P] * (T_pad // P)
    if T_pad % P:
        m_sizes.append(T_pad % P)
    MO = len(m_sizes)
    m_starts = [sum(m_sizes[:i]) for i in range(MO)]

    nc = bacc_mod.Bacc()
    xT = nc.dram_tensor("xT", [D, T_pad], bf16, kind="ExternalInput")
    w = nc.dram_tensor("w", [D, O], bf16, kind="ExternalInput")
    bias = nc.dram_tensor("bias", [P, O], f32, kind="ExternalInput")
    out = nc.dram_tensor("out", [T_pad, O], bf16, kind="ExternalOutput")

    xT_t = xT[:, :].rearrange("(ko p) t -> p ko t", p=P)
    w_t = w[:, :].rearrange("(ko p) o -> p ko o", p=P)

    # Tile schedule: (m, n) psum groups. Pass A holds 8 groups (all 8
    # PSUM banks) and runs k-outer: its ~1.73 us per-k-step burn rate
    # stays above the ~1.1 us/slice 3-queue DMA delivery, so the PE
    # never stalls once started. Pass B's two groups take the banks of
    # the first two pass-A groups, which are drained first (on separate
    # engines when the bias is all-zero, so both free ~0.7 us in).
    passA = [(m, 0) for m in range(MO)] + [(m, 1) for m in range(min(3, MO))]
    passA = passA[:8]
    passB = [(m, n) for n in range(NO) for m in range(MO) if (m, n) not in passA]

    with tile.TileContext(nc) as tc:
        with (
            tc.tile_pool(name="resident", bufs=1) as rpool,
            tc.tile_pool(name="psum", bufs=8, space="PSUM") as psum_pool,
            tc.tile_pool(name="obuf", bufs=MO) as opool,
        ):
            ps = {
                mn: psum_pool.tile(
                    [m_sizes[mn[0]], N_TILE], f32, tag="ps", name=f"ps_{mn[0]}_{mn[1]}"
                )
                for mn in passA
            }
            # HAM warm-up: dummy matmuls lift the PE clock gate to 8/8
            # before the real stream starts. Each bass-level warm matmul
            # lowers to 2 MATMUL instructions (measured), so 12 calls =
            # ~2.6 us of PE activity. They target the last pass-A psum
            # group as throwaway singleton groups — the real k=0 matmul
            # (start=True) clears the bank, so no extra bank is burned.
            warm_sb = rpool.tile([P, 64], f32, tag="warm")
            nc.gpsimd.memset(warm_sb[:], 0.0)
            warm_tgt = ps[passA[-1]]
            for i in range(12):
                nc.tensor.matmul(
                    warm_tgt[:64, :64],
                    lhsT=warm_sb[:, :64],
                    rhs=warm_sb[:, :64],
                    start=True,
                    stop=True,
                )
            # Input loads: one DMA per k-slice (x [128, T_pad], w
            # [128, O], both contiguous bf16), alternated across the two
            # HWDGE queues so slice k lands ~k * 1.1 us in — matching the
            # PE's ~1.7 us per k-step burn rate. bias arrives host-tiled
            # as [128, O] and is issued LAST on the scalar queue, so its
            # 512 KB transfers after all x/w slices (it is only needed at
            # the pass-A drain ~6 us later).
            bias_sb = rpool.tile([P, O], f32, tag="bias")
            x_sb = []
            w_sb = []
            # Slice loads over three issuing engines (two HWDGE queues +
            # gpsimd SWDGE). Queue spin-up is staggered (~0.6-2.2 us
            # between the sync queue's first byte and the others'), and
            # a queue running alone moves bytes at near-full HBM rate —
            # so k=0 and k=1 ride the early sync queue back-to-back
            # (ready ~9.4/~10.5 us, right as warm-up ends), while later
            # slices spread across all three queues. k=7 stays off the
            # slow-spin-up gpsimd queue.
            per_k_queue = {
                0: nc.sync,
                1: nc.sync,
                2: nc.scalar,
                3: nc.gpsimd,
                4: nc.sync,
                5: nc.scalar,
                6: nc.gpsimd,
                7: nc.scalar,
            }
            for k in range(KO):
                xt = rpool.tile([P, T_pad], bf16, tag=f"x{k}")
                wt = rpool.tile([P, O], bf16, tag=f"w{k}")
                q = per_k_queue[k]
                q.dma_start(wt[:], w_t[:, k, :])
                q.dma_start(xt[:], xT_t[:, k, :])
                x_sb.append(xt)
                w_sb.append(wt)
            if not bias_is_zero:
                nc.scalar.dma_start(bias_sb[:], bias[:, :])

            def x_ap(k, m):
                return x_sb[k][:, m_starts[m] : m_starts[m] + m_sizes[m]]

            obufs = [
                opool.tile([P, O], bf16, tag="ot", name=f"ot{m}")
                for m in range(MO)
            ]
            out_written = {m: 0 for m in range(MO)}
            split_out_ms = {m for (m, n) in passB}

            drain_idx = [0]

            def drain(mn):
                m, n = mn
                dst = obufs[m][: m_sizes[m], ts(n, N_TILE)]
                # With an all-zero bias the psum->obuf move is a pure
                # copy, which the scalar (ACT) engine can also do —
                # alternate DVE/ACT so the ~0.67 us-per-tile drain runs
                # two-wide. (gpsimd on a PSUM source fails NEFF compile;
                # ACT's bias operand is per-partition only, hence the
                # zero-bias specialization.)
                if bias_is_zero:
                    if drain_idx[0] % 2 == 0:
                        nc.vector.tensor_copy(dst, ps[mn][:])
                    else:
                        nc.scalar.copy(dst, ps[mn][:])
                else:
                    nc.vector.tensor_add(
                        dst,
                        ps[mn][:],
                        bias_sb[: m_sizes[m], ts(n, N_TILE)],
                    )
                drain_idx[0] += 1
                out_written[m] += 1
                eng = nc.sync if m % 2 == 0 else nc.scalar
                if m in split_out_ms:
                    # This m-tile's n-halves complete a pass apart —
                    # ship each half as soon as it drains (strided
                    # [m, 512] slab, 1 KB rows) instead of holding the
                    # early half hostage to the pass-B tail.
                    eng.dma_start(
                        out[m_starts[m] : m_starts[m] + m_sizes[m], ts(n, N_TILE)],
                        obufs[m][: m_sizes[m], ts(n, N_TILE)],
                    )
                elif out_written[m] == NO:
                    eng.dma_start(
                        out[m_starts[m] : m_starts[m] + m_sizes[m], :],
                        obufs[m][: m_sizes[m], :],
                    )

            for k in range(KO):
                for mn in passA:
                    nc.tensor.matmul(
                        ps[mn][:],
                        lhsT=x_ap(k, mn[0]),
                        rhs=w_sb[k][:, ts(mn[1], N_TILE)],
                        start=(k == 0),
                        stop=(k == KO - 1),
                    )
            for mn in passA:
                drain(mn)
            for mn in passB:
                ps[mn] = psum_pool.tile(
                    [m_sizes[mn[0]], N_TILE], f32, tag="ps", name=f"ps_{mn[0]}_{mn[1]}"
                )
                for k in range(KO):
                    nc.tensor.matmul(
                        ps[mn][:],
                        lhsT=x_ap(k, mn[0]),
                        rhs=w_sb[k][:, ts(mn[1], N_TILE)],
                        start=(k == 0),
                        stop=(k == KO - 1),
                    )
                drain(mn)
    nc.finalize()
    return nc


def kernel(x, category_id, weight, bias):
    global LAST_RESULTS
    x = np.asarray(x)
    category_id = np.asarray(category_id)
    weight = np.asarray(weight, dtype=np.float32)
    bias = np.ascontiguousarray(np.asarray(bias), dtype=np.float32)

    orig_shape = x.shape
    D = orig_shape[-1]
    C, _, O = weight.shape
    assert C == N_CORES and D % P == 0 and O % N_TILE == 0

    T = int(np.prod(orig_shape[:-1]))
    x_flat = np.ascontiguousarray(x.reshape(T, D), dtype=np.float32)
    cid = category_id.reshape(T).astype(np.int64)

    idx_per_c = [np.flatnonzero(cid == c) for c in range(C)]
    counts = [len(ix) for ix in idx_per_c]
    T_pad = max(32, -(-max(counts) // 32) * 32)  # multiple of 32 (PE col-group)

    bias_is_zero = not np.any(bias)
    key = (T_pad, D, O, bias_is_zero)
    if key not in _nc_cache:
        _nc_cache[key] = _build_nc(T_pad, D, O, bias_is_zero)
    nc = _nc_cache[key]

    w_bf16 = weight.astype(BF16)
    in_maps = []
    for c in range(C):
        xcT = np.zeros((D, T_pad), dtype=BF16)
        xcT[:, : counts[c]] = x_flat[idx_per_c[c]].T.astype(BF16)
        in_maps.append(
            {
                "xT": xcT,
                "w": w_bf16[c],
                "bias": np.ascontiguousarray(
                    np.broadcast_to(bias[c : c + 1], (P, O))
                ),
            }
        )

    res = run_bass_kernel_spmd(nc, in_maps, list(range(N_CORES)))
    LAST_RESULTS = res

    out_flat = np.empty((T, O), dtype=np.float32)
    for c in range(C):
        out_flat[idx_per_c[c]] = res.results[c]["out"][: counts[c]].astype(
            np.float32
        )
    return out_flat.reshape(*orig_shape[:-1], O)


# revision 37
# speedup vs baseline: 1.0458x; 1.0458x over previous
"""CategorySpecificLinear Trainium2 kernel.

out[t] = x[t] @ weight[category_id[t]] + bias[category_id[t]]

Strategy: expert-parallel over the 8 categories (C == n_cores == 8).
Host routes tokens by category, transposes each category's token block
to [D, T_pad] and casts x/w to bf16 (fp32 accumulate in PSUM keeps the
rel err ~3e-3, far under the 2e-2 gate). Core c computes
    out = xT.T @ w + bias    (out in bf16, host casts back to fp32)

vs the fp32r baseline (44.2 us -> 34.7 us measured):
  - bf16 halves HBM traffic (3.4 MB/core vs 9.2) and matmul cost
    (N=512 warm matmul spacing 216 ns vs 231, LDWEIGHTS ~95 ns).
  - pass A holds 8 (m, n) psum groups (all banks) k-outer, so its
    ~1.73 us per-k-step burn rate stays above the ~1.1 us/slice DMA
    delivery and the PE runs gap-free; pass B's 2 groups reuse the
    first-drained banks. ~12 bass warm-up matmuls (~2.6 us of PE
    activity) lift the HAM clock gate to 8/8 right as k=0 lands.
  - x/w slice loads rotate over 3 issuing engines (2 HWDGE + SWDGE) —
    with 2 queues the ~0.65 us per-DMA issue cost, not HBM bandwidth,
    limits delivery. The 512 KB host-tiled bias load is issued last so
    it transfers after all x/w slices (needed ~6 us later); when bias
    is all zero it is skipped and the psum->obuf drain alternates
    DVE tensor_copy / ACT copy to run two-wide.
  - out is one contiguous [m, 1024] bf16 DMA per m-tile, small
    remainder tile last, so the post-matmul tail is ~2 us.
Fixed costs outside kernel control: ~1 us framework head and ~8.6 us
postamble (per-semaphore wind-down emitted by the NEFF wrapper).

Also tried, no better: a flipped orientation (psum = w_slice.T @ x,
token remainder as narrow matmuls sharing stationary weights) measured
the same within noise (34.9 us); k-pair-batched loads, w0 split across
queues, and same-queue k0 loads all regressed (DMA issue/packet-rate
effects dominate); gpsimd tensor ops on a PSUM source fail NEFF
compile.
"""

import contextlib
import ctypes
import os
import sys
import types

import numpy as np
import ml_dtypes

sys.path.insert(0, "/opt/trn_rl_repo")

BF16 = np.dtype(ml_dtypes.bfloat16)


def _ensure_ntff_hook():
    """Provide antenv.axon_hooks if the image lacks it.

    concourse.bass_utils imports antenv.axon_hooks.get_axon_ntff_profile_hook
    when trace=True under axon; some agent images don't ship that module, in
    which case the boot's NTFF hook registration silently degrades and the
    import in bass_utils crashes. Recreate the slim ctypes hook here
    (mirrors trn_agent_boot.trn_boot._ntff_profile_via_ctypes).
    """
    try:
        import antenv.axon_hooks  # noqa: F401

        return
    except ImportError:
        pass

    so_path = "/opt/axon/libaxon_pjrt.so"
    hook = None
    if os.path.exists(so_path):
        lib = ctypes.CDLL(so_path)
        if hasattr(lib, "axon_start_nrt_profile"):
            lib.axon_start_nrt_profile.argtypes = [
                ctypes.POINTER(ctypes.c_int64),
                ctypes.c_size_t,
            ]
            lib.axon_start_nrt_profile.restype = ctypes.c_int64
            lib.axon_stop_nrt_profile.argtypes = [ctypes.c_char_p]
            lib.axon_stop_nrt_profile.restype = ctypes.c_int64

            @contextlib.contextmanager
            def hook(output_dir, device_ids):
                import jax

                jax.devices()
                if device_ids:
                    ids = (ctypes.c_int64 * len(device_ids))(*device_ids)
                    rc = lib.axon_start_nrt_profile(ids, len(device_ids))
                else:
                    rc = lib.axon_start_nrt_profile(None, 0)
                if rc != 0:
                    raise RuntimeError(f"axon_start_nrt_profile rc={rc}")
                try:
                    yield
                finally:
                    n = lib.axon_stop_nrt_profile(str(output_dir).encode())
                    if n <= 0:
                        print(
                            f"ntff profile: rc={n} writing {output_dir}",
                            file=sys.stderr,
                        )

    mod = types.ModuleType("antenv.axon_hooks")
    _state = {"hook": hook}
    mod.set_axon_ntff_profile_hook = lambda h: _state.__setitem__("hook", h)
    mod.get_axon_ntff_profile_hook = lambda: _state["hook"]
    sys.modules["antenv.axon_hooks"] = mod
    try:
        import antenv

        antenv.axon_hooks = mod
    except ImportError:
        pass


_ensure_ntff_hook()

import concourse.bass as bass
import concourse.bacc as bacc_mod
import concourse.mybir as mybir
import concourse.tile as tile
from concourse.bass import ts
from concourse.bass_utils import run_bass_kernel_spmd

N_CORES = 8
P = 128
N_TILE = 512  # one fp32 PSUM bank

_nc_cache = {}
LAST_RESULTS = None  # BassKernelResults of the most recent run (for test.py)


def _build_nc(T_pad: int, D: int, O: int, bias_is_zero: bool = False):
    KO = D // P
    NO = O // N_TILE
    bf16 = mybir.dt.bfloat16
    f32 = mybir.dt.float32

    # m-tiles: full 128-row tiles plus one remainder tile (multiple of 32)
    m_sizes = [P] * (T_pad // P)
    if T_pad % P:
        m_sizes.append(T_pad % P)
    MO = len(m_sizes)
    m_starts = [sum(m_sizes[:i]) for i in range(MO)]

    nc = bacc_mod.Bacc()
    xT = nc.dram_tensor("xT", [D, T_pad], bf16, kind="ExternalInput")
    w = nc.dram_tensor("w", [D, O], bf16, kind="ExternalInput")
    bias = nc.dram_tensor("bias", [P, O], f32, kind="ExternalInput")
    out = nc.dram_tensor("out", [T_pad, O], bf16, kind="ExternalOutput")

    xT_t = xT[:, :].rearrange("(ko p) t -> p ko t", p=P)
    w_t = w[:, :].rearrange("(ko p) o -> p ko o", p=P)

    # Tile schedule: (m, n) psum groups. Pass A holds 8 groups (all 8
    # PSUM banks) and runs k-outer: its ~1.73 us per-k-step burn rate
    # stays above the ~1.1 us/slice 3-queue DMA delivery, so the PE
    # never stalls once started. Pass B's two groups take the banks of
    # the first two pass-A groups, which are drained first (on separate
    # engines when the bias is all-zero, so both free ~0.7 us in).
    passA = [(m, 0) for m in range(MO)] + [(m, 1) for m in range(min(3, MO))]
    passA = passA[:8]
    passB = [(m, n) for n in range(NO) for m in range(MO) if (m, n) not in passA]

    with tile.TileContext(nc) as tc:
        with (
            tc.tile_pool(name="resident", bufs=1) as rpool,
            tc.tile_pool(name="psum", bufs=8, space="PSUM") as psum_pool,
            tc.tile_pool(name="obuf", bufs=MO) as opool,
        ):
            ps = {
                mn: psum_pool.tile(
                    [m_sizes[mn[0]], N_TILE], f32, tag="ps", name=f"ps_{mn[0]}_{mn[1]}"
                )
                for mn in passA
            }
            # HAM warm-up: dummy matmuls lift the PE clock gate to 8/8
            # before the real stream starts. Each bass-level warm matmul
            # lowers to 2 MATMUL instructions (measured), so 12 calls =
            # ~2.6 us of PE activity. They target the last pass-A psum
            # group as throwaway singleton groups — the real k=0 matmul
            # (start=True) clears the bank, so no extra bank is burned.
            warm_sb = rpool.tile([P, 64], f32, tag="warm")
            nc.gpsimd.memset(warm_sb[:], 0.0)
            warm_tgt = ps[passA[-1]]
            for i in range(12):
                nc.tensor.matmul(
                    warm_tgt[:64, :64],
                    lhsT=warm_sb[:, :64],
                    rhs=warm_sb[:, :64],
                    start=True,
                    stop=True,
                )
            # Input loads: one DMA per k-slice (x [128, T_pad], w
            # [128, O], both contiguous bf16), alternated across the two
            # HWDGE queues so slice k lands ~k * 1.1 us in — matching the
            # PE's ~1.7 us per k-step burn rate. bias arrives host-tiled
            # as [128, O] and is issued LAST on the scalar queue, so its
            # 512 KB transfers after all x/w slices (it is only needed at
            # the pass-A drain ~6 us later).
            bias_sb = rpool.tile([P, O], f32, tag="bias")
            x_sb = []
            w_sb = []
            # Rotate x/w slice loads over three issuing engines (two
            # HWDGE queues + gpsimd SWDGE): each ~0.65 us issue is the
            # delivery bottleneck with only two queues. k=0 stays on the
            # HWDGE queues (lower first-byte latency), with w0 — the
            # larger transfer gating the first k-step — leading on the
            # sync queue, which starts moving bytes ~0.6 us before the
            # scalar queue. (Co-queueing each k's x+w pair, or k0+k1
            # both on sync, measured 2-5 us WORSE: queues share HBM at
            # ~1/3 rate once all are active, so serializing a k-step's
            # two slices on one queue delays it past the PE's need.)
            queues = [nc.sync, nc.scalar, nc.gpsimd]
            # Prime each DMA queue with a throwaway 4 KB read first: the
            # first packet on a queue lags its doorbell by ~0.7-2.2 us
            # (ring spin-up), so absorbing that on a dummy transfer lets
            # the real k=0/k=1 slices flow at full rate immediately.
            for qi, q in enumerate(queues):
                pr = rpool.tile([P, 16], bf16, tag=f"prime{qi}")
                q.dma_start(pr[:], xT_t[:, 0, :16])
            for k in range(KO):
                xt = rpool.tile([P, T_pad], bf16, tag=f"x{k}")
                wt = rpool.tile([P, O], bf16, tag=f"w{k}")
                xq = queues[(2 * k) % 3]
                wq = queues[(2 * k + 1) % 3]
                if k == 0:
                    xq, wq = wq, xq
                xq.dma_start(xt[:], xT_t[:, k, :])
                wq.dma_start(wt[:], w_t[:, k, :])
                x_sb.append(xt)
                w_sb.append(wt)
            if not bias_is_zero:
                nc.scalar.dma_start(bias_sb[:], bias[:, :])

            def x_ap(k, m):
                return x_sb[k][:, m_starts[m] : m_starts[m] + m_sizes[m]]

            obufs = [
                opool.tile([P, O], bf16, tag="ot", name=f"ot{m}")
                for m in range(MO)
            ]
            out_written = {m: 0 for m in range(MO)}
            split_out_ms = {m for (m, n) in passB}

            drain_idx = [0]

            def drain(mn):
                m, n = mn
                dst = obufs[m][: m_sizes[m], ts(n, N_TILE)]
                # With an all-zero bias the psum->obuf move is a pure
                # copy, which the scalar (ACT) engine can also do —
                # alternate DVE/ACT so the ~0.67 us-per-tile drain runs
                # two-wide. (gpsimd on a PSUM source fails NEFF compile;
                # ACT's bias operand is per-partition only, hence the
                # zero-bias specialization.)
                if bias_is_zero:
                    if drain_idx[0] % 2 == 0:
                        nc.vector.tensor_copy(dst, ps[mn][:])
                    else:
                        nc.scalar.copy(dst, ps[mn][:])
                else:
                    nc.vector.tensor_add(
                        dst,
                        ps[mn][:],
                        bias_sb[: m_sizes[m], ts(n, N_TILE)],
                    )
                drain_idx[0] += 1
                out_written[m] += 1
                eng = nc.sync if m % 2 == 0 else nc.scalar
                if m in split_out_ms:
                    # This m-tile's n-halves complete a pass apart —
                    # ship each half as soon as it drains (strided
                    # [m, 512] slab, 1 KB rows) instead of holding the
                    # early half hostage to the pass-B tail.
                    eng.dma_start(
                        out[m_starts[m] : m_starts[m] + m_sizes[m], ts(n, N_TILE)],
                        obufs[m][: m_sizes[m], ts(n, N_TILE)],
                    )
                elif out_written[m] == NO:
                    eng.dma_start(
                        out[m_starts[m] : m_starts[m] + m_sizes[m], :],
                        obufs[m][: m_sizes[m], :],
                    )

            for k in range(KO):
                for mn in passA:
                    nc.tensor.matmul(
                        ps[mn][:],
                        lhsT=x_ap(k, mn[0]),
                        rhs=w_sb[k][:, ts(mn[1], N_TILE)],
                        start=(k == 0),
                        stop=(k == KO - 1),
                    )
            for mn in passA:
                drain(mn)
            for mn in passB:
                ps[mn] = psum_pool.tile(
                    [m_sizes[mn[0]], N_TILE], f32, tag="ps", name=f"ps_{mn[0]}_{mn[1]}"
                )
                for k in range(KO):
                    nc.tensor.matmul(
                        ps[mn][:],
                        lhsT=x_ap(k, mn[0]),
                        rhs=w_sb[k][:, ts(mn[1], N_TILE)],
                        start=(k == 0),
                        stop=(k == KO - 1),
                    )
                drain(mn)
    nc.finalize()
    return nc


def kernel(x, category_id, weight, bias):
    global LAST_RESULTS
    x = np.asarray(x)
    category_id = np.asarray(category_id)
    weight = np.asarray(weight, dtype=np.float32)
    bias = np.ascontiguousarray(np.asarray(bias), dtype=np.float32)

    orig_shape = x.shape
    D = orig_shape[-1]
    C, _, O = weight.shape
    assert C == N_CORES and D % P == 0 and O % N_TILE == 0

    T = int(np.prod(orig_shape[:-1]))
    x_flat = np.ascontiguousarray(x.reshape(T, D), dtype=np.float32)
    cid = category_id.reshape(T).astype(np.int64)

    idx_per_c = [np.flatnonzero(cid == c) for c in range(C)]
    counts = [len(ix) for ix in idx_per_c]
    T_pad = max(32, -(-max(counts) // 32) * 32)  # multiple of 32 (PE col-group)

    bias_is_zero = not np.any(bias)
    key = (T_pad, D, O, bias_is_zero)
    if key not in _nc_cache:
        _nc_cache[key] = _build_nc(T_pad, D, O, bias_is_zero)
    nc = _nc_cache[key]

    w_bf16 = weight.astype(BF16)
    in_maps = []
    for c in range(C):
        xcT = np.zeros((D, T_pad), dtype=BF16)
        xcT[:, : counts[c]] = x_flat[idx_per_c[c]].T.astype(BF16)
        in_maps.append(
            {
                "xT": xcT,
                "w": w_bf16[c],
                "bias": np.ascontiguousarray(
                    np.broadcast_to(bias[c : c + 1], (P, O))
                ),
            }
        )

    res = run_bass_kernel_spmd(nc, in_maps, list(range(N_CORES)))
    LAST_RESULTS = res

    out_flat = np.empty((T, O), dtype=np.float32)
    for c in range(C):
        out_flat[idx_per_c[c]] = res.results[c]["out"][: counts[c]].astype(
            np.float32
        )
    return out_flat.reshape(*orig_shape[:-1], O)
